# revision 6
# baseline (speedup 1.0000x reference)
"""Trainium2 Bass kernel: batched single-channel 7x7 conv2d (stride 1, pad 3).

Strategy (v4: warm-paced 8-way 64x32 PE tiling, pre-tiled host layouts)
----------------------------------------------------------------------
Pure data parallel over batch: 64 images -> 8 cores x 8 images.

The 7-tap vertical band Toeplitz matmul populates only a 7-wide band of
the stationary operand, so a full 128x128 matmul wastes ~94% of the PE
array. The array is reconfigured into 8 independent 64x32 subarrays
(tile_position): each computes a (K=38 input rows, M=32 output rows)
band matmul. M=32 exactly fills a 32-partition PSUM column group, so
PSUM banks pack 128 consecutive output rows gap-free.

Pass group = 8 windows (4 consecutive 32-row output blocks per SBUF
half) x 2 column blocks = 16 tasks on 8 subarray positions. PSUM bank
(b, h) = windows of half h at column block b = [128 rows x 512 cols].

v4 changes vs v3 (trace-driven):
- HAM warmth: the v3 trace showed the PE oscillating between 2.4 GHz
  and 1.2 GHz (tiled matmuls do not register as PE-busy to the HAM
  clock gate, so the MID window re-throttles mid-run; the every-8-group
  rewarm bursts only partially recovered). v4 issues 2 full-array
  (HAM-visible) N=512 dummy matmuls per group (~1.7us spacing < 3.4us
  MID window) instead of 8 tiled pacing dummies + 20-matmul rewarms.
- DMA descriptor fattening via host pre-tiling. Input is staged k-major
  [b, k=40, w=32, 1032]: each half-group load is one descriptor per
  partition (8256 B) instead of 4x2064 B. Output DRAM is pre-tiled
  [32 groups, 128, 2048] so each store is 128 contiguous 4 KiB
  descriptors; the host un-tiles.

I/O bf16 both ways: ~21 MB in + 16.8 MB out per core. Host upcasts the
output to fp32.
"""

import numpy as np
import ml_dtypes
from contextlib import ExitStack

import concourse.bass as bass
import concourse.tile as tile
from concourse import bacc, mybir
from concourse.bass_utils import run_bass_kernel_spmd

N_CORES = 8
B, H, W_IMG = 64, 1024, 1024
B_LOC = B // N_CORES
KS, PAD = 7, 3
WP = 1032               # padded row width (3 left, 5 right zeros)
WIN_K = 38              # input rows feeding one 32-row output block
# 40, not 38: DMA engine spreading wants a multiple-of-8 partition count.
K_LOAD = 40
WIN_M = 32              # output rows per window
COL_BLOCK = 512
N_WIN = H // WIN_M      # 32 windows per image
WINS_PER_GROUP = 8      # 4 per SBUF half
GROUPS_PER_IMG = N_WIN // WINS_PER_GROUP  # 4


def build_toeplitz(w7, np_dt):
    """Band weights [128, 7*32 + 512] bf16, replicated in partition halves.

    Slice [64h : 64h+38, 32v : 32v+32][k, m] = W[k-m, v] (0 outside the
    band). Trailing 512-col zero block feeds the PE warm/pace matmuls.
    """
    t = np.zeros((128, KS * WIN_M), dtype=np.float32)
    k = np.arange(WIN_K)[:, None]
    m = np.arange(WIN_M)[None, :]
    u = k - m
    mask = (u >= 0) & (u < KS)
    uu = np.clip(u, 0, KS - 1)
    for v in range(KS):
        band = np.where(mask, w7[uu, v], 0.0)
        t[0:WIN_K, v * WIN_M : (v + 1) * WIN_M] = band
        t[64 : 64 + WIN_K, v * WIN_M : (v + 1) * WIN_M] = band
    t = np.concatenate([t, np.zeros((128, COL_BLOCK), np.float32)], axis=1)
    return np.ascontiguousarray(t.astype(np_dt))


def build_tiled_input(X, np_dt):
    """Pre-tiled k-major input [B, K_LOAD, N_WIN, WP] bf16.

    xp2[b, k, w, 3+c] = X[b, 32w + k - 3, c] (zeros outside), so a
    half-group load is one contiguous 4*WP run per partition.
    """
    Xb = X.astype(np_dt)
    k = np.arange(K_LOAD)[:, None]
    w = np.arange(N_WIN)[None, :]
    r = WIN_M * w + k - PAD                  # [K_LOAD, N_WIN]
    valid = (r >= 0) & (r < H)
    idx = np.clip(r, 0, H - 1)
    xp2 = np.zeros((B, K_LOAD, N_WIN, WP), dtype=np_dt)
    tmp = Xb[:, idx, :]                      # [B, K_LOAD, N_WIN, W_IMG]
    tmp[:, ~valid] = 0
    xp2[:, :, :, PAD : PAD + W_IMG] = tmp
    return xp2


def build_program(b_loc, in_dt=mybir.dt.bfloat16):
    n_toep = KS * WIN_M
    n_groups = b_loc * GROUPS_PER_IMG

    nc = bacc.Bacc("TRN2", target_bir_lowering=False, debug=False)
    x_d = nc.dram_tensor(
        "x", [b_loc, K_LOAD, N_WIN, WP], in_dt, kind="ExternalInput"
    ).ap()
    t_d = nc.dram_tensor(
        "toep", [128, n_toep + COL_BLOCK], in_dt, kind="ExternalInput"
    ).ap()
    y_d = nc.dram_tensor(
        "y", [n_groups, 128, 4 * COL_BLOCK], in_dt, kind="ExternalOutput"
    ).ap()

    with tile.TileContext(nc) as tc, ExitStack() as ctx:
        wpool = ctx.enter_context(tc.tile_pool(name="wpool", bufs=1))
        inpool = ctx.enter_context(tc.tile_pool(name="inpool", bufs=4))
        outpool = ctx.enter_context(tc.tile_pool(name="outpool", bufs=4))
        pspool = ctx.enter_context(tc.tile_pool(name="pspool", bufs=8, space="PSUM"))

        wt = wpool.tile([128, n_toep + COL_BLOCK], in_dt, name="wt")
        nc.sync.dma_start(wt[:], t_d[:])

        # Bridge the HAM un-throttle (needs ~3.4us of sustained
        # full-array activity) and the first groups' input loads.
        warm = wt[:, n_toep:]
        wps = pspool.tile([128, COL_BLOCK], mybir.dt.float32, name="wps", tag="ps")
        N_WARM = 20
        for i in range(N_WARM):
            nc.tensor.matmul(
                wps[:], warm[:, :128], warm[:], start=(i == 0), stop=(i == N_WARM - 1)
            )

        def burst(n=14):
            # Tiled matmuls do not register as PE-busy to the HAM clock
            # gate, and isolated full-array matmuls cannot un-throttle it
            # (needs ~3.4us of SUSTAINED full-array activity): the v4
            # trace showed 427ns (1.2 GHz) pace matmuls all run long. The
            # v3 trace showed a 20-matmul burst restores 2.4 GHz for
            # ~15-30us of tiled work. So: sustained bursts on a ~13us
            # cadence (every 4 groups).
            for i in range(n):
                nc.tensor.matmul(
                    wps[:], warm[:, :128], warm[:], start=(i == 0), stop=(i == n - 1)
                )

        order = [PAD] + [v for v in range(KS) if v != PAD]

        def preload(g):
            # One k-major load per SBUF half: [40 partitions, 4*WP] with a
            # single contiguous 8256 B descriptor per partition. Emitted 2
            # groups ahead of use so the loads sit in the scalar/sync
            # queues BEFORE the current group's store (FIFO queues).
            bi = g // GROUPS_PER_IMG
            w0 = WINS_PER_GROUP * (g % GROUPS_PER_IMG)
            wtile = inpool.tile([128, 4 * WP], in_dt, name="wtile", tag="in")
            for h in range(2):
                src = bass.AP(
                    x_d.tensor,
                    (bi * K_LOAD * N_WIN + w0 + 4 * h) * WP,
                    [[N_WIN * WP, K_LOAD], [1, 4 * WP]],
                )
                # SWDGE: the gpsimd descriptor-generation interleaves the
                # 40-partition halves across all 16 SDMA engines (HWDGE
                # packs them onto 5 consecutive engines per queue; loads
                # then bottleneck on 10 of 16 engines). DMA-only probe:
                # 147us (HWDGE) -> 134us (SWDGE loads).
                nc.gpsimd.dma_start(wtile[64 * h : 64 * h + K_LOAD, :], src)
            return wtile

        wtiles = {g: preload(g) for g in range(min(2, n_groups))}
        for g in range(n_groups):
            if g and g % 2 == 0:
                burst(10)
            if g + 2 < n_groups:
                wtiles[g + 2] = preload(g + 2)
            wtile = wtiles.pop(g)

            # PSUM bank (cb, h) <- half h's 4 windows at column block cb.
            pss = [
                pspool.tile([128, COL_BLOCK], mybir.dt.float32, name="ps", tag="ps")
                for _ in range(4)
            ]
            for vi, v in enumerate(order):
                # cb outer / s inner: consecutive matmuls target different
                # subarrays so the 8 positions run concurrently.
                for cb in range(2):
                    for s in range(WINS_PER_GROUP):  # subarray position
                        h, wq = s // 4, s % 4
                        nc.tensor.matmul(
                            pss[2 * cb + h][32 * wq : 32 * wq + WIN_M, :],
                            wt[64 * h : 64 * h + WIN_K, v * WIN_M : (v + 1) * WIN_M],
                            wtile[
                                64 * h : 64 * h + WIN_K,
                                WP * wq + COL_BLOCK * cb + v :
                                WP * wq + COL_BLOCK * cb + v + COL_BLOCK,
                            ],
                            start=(vi == 0),
                            stop=(vi == len(order) - 1),
                            tile_position=(64 * h, 32 * wq),
                        )

            # Evacuate 4 banks into one [128, 2048] bf16 tile laid out
            # (h, cb, c) so each partition's 4 KiB is one contiguous DRAM
            # run of output row (256*(g%4) + 128h + p).
            ot = outpool.tile([128, 4 * COL_BLOCK], in_dt, name="ot", tag="ot")
            for cb in range(2):
                for h in range(2):
                    dst = ot[:, 2 * h * COL_BLOCK + cb * COL_BLOCK :
                               2 * h * COL_BLOCK + (cb + 1) * COL_BLOCK]
                    if h == 0:
                        nc.vector.tensor_copy(dst, pss[2 * cb + h][:])
                    else:
                        nc.scalar.copy(dst, pss[2 * cb + h][:])
            dst = bass.AP(
                y_d.tensor,
                g * 128 * 4 * COL_BLOCK,
                [[4 * COL_BLOCK, 128], [1, 4 * COL_BLOCK]],
            )
            nc.sync.dma_start(dst, ot[:])

    nc.compile()
    return nc


_CACHE = {}


def _get_program(b_loc, in_dt):
    key = (b_loc, in_dt)
    if key not in _CACHE:
        _CACHE[key] = build_program(b_loc, in_dt=in_dt)
    return _CACHE[key]


IN_DT = mybir.dt.bfloat16


def kernel(X, W, _trace=False, _trace_dir=None):
    X = np.asarray(X, dtype=np.float32)
    W = np.asarray(W, dtype=np.float32)
    assert X.shape == (B, H, W_IMG) and W.shape == (KS, KS)

    nc = _get_program(B_LOC, IN_DT)
    np_dt = mybir.dt.np(IN_DT)
    xp2 = build_tiled_input(X, np_dt)
    toep = build_toeplitz(W, np_dt)
    in_maps = [
        {"x": xp2[c * B_LOC : (c + 1) * B_LOC], "toep": toep}
        for c in range(N_CORES)
    ]
    res = run_bass_kernel_spmd(
        nc, in_maps, list(range(N_CORES)), trace=_trace, tmpdir=_trace_dir
    )
    outs = []
    for c in range(N_CORES):
        y2 = res.results[c]["y"]  # [32, 128, 2048] bf16
        y2 = y2.reshape(B_LOC, GROUPS_PER_IMG, 128, 2, 1024)
        y2 = y2.transpose(0, 1, 3, 2, 4).reshape(B_LOC, H, W_IMG)
        outs.append(y2.astype(np.float32))
    out = np.concatenate(outs, axis=0)
    if _trace:
        return out, res
    return out
